# revision 20
# baseline (speedup 1.0000x reference)
"""Distributed Trainium2 kernel for causal GQA attention with RoPE.

Tensor-parallel over heads across 8 NeuronCores: core c owns q heads
4c..4c+3 and kv head c.  Activations are kept transposed ([dim, seq])
so every matmul contracts over the partition axis:

  phase 1: qkvT = wqkv_c @ x.T   (RoPE fused into the PSUM eviction;
           v transposed back to [seq, dim] via PE transposes, pipelined
           per seq chunk)
  phase 2: per head, causal attention with TRANSPOSED scores:
           S^T[k,q] = kT_tile^T @ qT  ->  masked exp -> P^T in SBUF
           (no PE transposes of P needed: P^T is already the PV moving
           operand).  Row sums accumulate on VectorE (partition tree),
           reciprocal broadcast to all partitions via a PE outer
           product, folded into the PV eviction.
  phase 3: AllGather oT over cores in per-head HALVES (8 small
           collectives issued as soon as each half of each head's oT is
           ready), then out_cT = wo_c.T.T @ attn_allT.

Output per core is the transposed column shard [512, 2048] of the final
projection; the host concatenates and transposes.
"""

import math
import sys

if "/opt/trn_rl_repo" not in sys.path:
    sys.path.insert(0, "/opt/trn_rl_repo")

from contextlib import ExitStack

import numpy as np
import ml_dtypes

import concourse.bacc as bacc
import concourse.bass_isa as bass_isa
import concourse.mybir as mybir
from concourse.tile import TileContext
from concourse.masks import make_identity
from concourse.bass_utils import run_bass_kernel_spmd

N_CORES = 8
H = 4096          # model dim
HD = 128          # head dim
QH = 4            # q heads per core
SCALE = 1.0 / math.sqrt(HD)
MASK_VAL = -1.0e5
DT = mybir.dt


def build_nc(S=2048, debug=False):
    KT = H // 128           # contraction tiles for both GEMMs
    NCH = max(1, S // 512)  # seq chunks of 512
    CH = S // NCH           # chunk size
    QT = S // 128           # q row tiles (== k tiles)
    NG = max(1, S // 512)   # phase-2 q groups of 512
    GW = S // NG            # group width
    M1 = QH + 2             # phase-1 output row tiles: 4 q heads, k, v
    DV = GW // 128          # diagonal variants per group

    nc = bacc.Bacc("TRN2", target_bir_lowering=False, debug=debug,
                   num_devices=N_CORES)
    xT = nc.declare_dram_parameter("xT", [H, S], DT.bfloat16, isOutput=False)
    wqkvT = nc.declare_dram_parameter("wqkvT", [H, 128 * M1], DT.bfloat16,
                                      isOutput=False)
    woT = nc.declare_dram_parameter("woT", [H, 128 * QH], DT.bfloat16,
                                    isOutput=False)
    cosT = nc.declare_dram_parameter("cosT", [HD, S], DT.float32,
                                     isOutput=False)
    sinTs = nc.declare_dram_parameter("sinTs", [HD, S], DT.float32,
                                      isOutput=False)
    out = nc.declare_dram_parameter("out", [128 * QH, S], DT.float32,
                                    isOutput=True)

    with TileContext(nc) as tc, ExitStack() as ctx:
        persist = ctx.enter_context(tc.tile_pool(name="persist", bufs=1))
        cos_sb = persist.tile([HD, S], DT.float32, name="cos_sb")
        sins_sb = persist.tile([HD, S], DT.float32, name="sins_sb")
        # qk_sb[0:4] = roped qT per head, qk_sb[4] = roped kT
        qk_sb = [persist.tile([128, S], DT.bfloat16, name=f"qk{m}")
                 for m in range(QH + 1)]
        vT_sb = persist.tile([128, S], DT.bfloat16, name="vT_sb")
        v_sb = persist.tile([128, S], DT.bfloat16, name="v_sb")
        oT_sb = [persist.tile([128, S], DT.bfloat16, name=f"oT{h}")
                 for h in range(QH)]
        ident = persist.tile([128, 128], DT.bfloat16, name="ident")
        make_identity(nc, ident[:])
        # transposed causal triangle for the diagonal 128x128 block of the
        # S^T chunks: keep (0) where q - k = y - x >= 0, else MASK_VAL.
        # bf16 so the mask-add matmul stays single-pass (fp32 moving
        # operands lower to a 2x LOW_HIGH pass on the PE).
        maskTri = persist.tile([128, 128], DT.bfloat16, name="maskTri")
        nc.gpsimd.memset(maskTri[:], 0.0)
        nc.gpsimd.affine_select(
            out=maskTri[:], in_=maskTri[:],
            compare_op=mybir.AluOpType.is_ge,
            fill=MASK_VAL,
            base=0,
            pattern=[[1, 128]],
            channel_multiplier=-1,
        )

        # ---------------- phase 1: qkv projection + rope -----------------
        with ExitStack() as s1, nc.named_scope("phase1_qkv"):
            vtp = s1.enter_context(tc.tile_pool(name="vtp", bufs=1,
                                                space="PSUM"))
            # warm the PE HAM clock gate while the first DMAs land
            warm = vtp.tile([128, 128], DT.bfloat16, name="warm", tag="vtp")
            for _ in range(72):
                nc.tensor.transpose(warm[:], ident[:], ident[:])

            wq_pool = s1.enter_context(tc.tile_pool(name="wqkv", bufs=1))
            # whole wqkvT, quarter DMAs interleaved with the x quarters below
            wq_big = wq_pool.tile([128, KT * 128 * M1], DT.bfloat16,
                                  name="wq_big")
            KG = KT // 4

            def load_wq_quarter(kg):
                nc.sync.dma_start(
                    out=wq_big[:, kg * KG * 128 * M1:
                               (kg + 1) * KG * 128 * M1].rearrange(
                        "p (k m) -> p k m", k=KG),
                    in_=wqkvT[kg * KG * 128:(kg + 1) * KG * 128, :].rearrange(
                        "(k p) m -> p k m", p=128))
            xpool = s1.enter_context(tc.tile_pool(name="xpool", bufs=2))
            acc1 = s1.enter_context(tc.tile_pool(name="acc1", bufs=1,
                                                 space="PSUM"))
            tmp1 = s1.enter_context(tc.tile_pool(name="tmp1", bufs=2))

            def v_transposes(cb):
                # v chunk back to [seq, dim] blocks (emission delayed one
                # chunk so the PE never waits on the scalar v eviction)
                for j in range(CH // 128):
                    jj = cb * (CH // 128) + j
                    js = slice(128 * jj, 128 * (jj + 1))
                    t = vtp.tile([128, 128], DT.bfloat16, name=f"vtp{jj}",
                                 tag="vtp")
                    nc.tensor.transpose(t[:], vT_sb[:, js], ident[:])
                    nc.scalar.copy(v_sb[:, js], t[:])

            for cb in range(NCH):
                ns = slice(CH * cb, CH * (cb + 1))
                accs = [acc1.tile([128, CH], DT.float32,
                                  name=f"acc1_{cb}_{m}", tag=f"acc{m}")
                        for m in range(M1)]
                # one DMA per seq chunk: free layout (k, n)
                xb = xpool.tile([128, KT * CH], DT.bfloat16,
                                name=f"xb_{cb}", tag="xb")
                for kg in range(4):
                    if cb == 0:
                        load_wq_quarter(kg)
                    nc.sync.dma_start(
                        out=xb[:, kg * KG * CH:(kg + 1) * KG * CH].rearrange(
                            "p (k n) -> p k n", k=KG),
                        in_=xT[kg * KG * 128:(kg + 1) * KG * 128,
                               ns].rearrange("(k p) n -> p k n", p=128))
                if cb == 0:
                    nc.sync.dma_start(out=cos_sb[:], in_=cosT[:])
                    nc.sync.dma_start(out=sins_sb[:], in_=sinTs[:])
                for k in range(KT):
                    for m in range(M1):
                        nc.tensor.matmul(accs[m][:],
                                         wq_big[:, (k * M1 + m) * 128:
                                                   (k * M1 + m + 1) * 128],
                                         xb[:, k * CH:(k + 1) * CH],
                                         start=(k == 0), stop=(k == KT - 1))
                if cb > 0:
                    v_transposes(cb - 1)
                for m in range(QH + 1):
                    # rope: out = acc*cos + swap_halves(acc)*sin_signed
                    tmp = tmp1.tile([128, CH], DT.float32,
                                    name=f"tmp_{cb}_{m}", tag="tmp")
                    nc.vector.tensor_tensor(out=tmp[0:64, :],
                                            in0=accs[m][64:128, :],
                                            in1=sins_sb[0:64, ns],
                                            op=mybir.AluOpType.mult)
                    nc.vector.tensor_tensor(out=tmp[64:128, :],
                                            in0=accs[m][0:64, :],
                                            in1=sins_sb[64:128, ns],
                                            op=mybir.AluOpType.mult)
                    nc.vector.tensor_tensor(out=qk_sb[m][:, ns],
                                            in0=accs[m][:],
                                            in1=cos_sb[:, ns],
                                            op=mybir.AluOpType.mult)
                    nc.vector.tensor_tensor(out=qk_sb[m][:, ns],
                                            in0=qk_sb[m][:, ns],
                                            in1=tmp[:],
                                            op=mybir.AluOpType.add)
                nc.scalar.copy(vT_sb[:, ns], accs[QH + 1][:])
            v_transposes(NCH - 1)

        # load wo early (no deps): issued on Sync right after phase-1
        # loads, lands long before the output projection needs it
        wo_pool = ctx.enter_context(tc.tile_pool(name="wop", bufs=1))
        wo_big = wo_pool.tile([128, KT * 128 * QH], DT.bfloat16,
                              name="wo_big")
        nc.sync.dma_start(
            out=wo_big[:].rearrange("p (k m) -> p k m", k=KT),
            in_=woT.rearrange("(k p) m -> p k m", p=128))

        # ---------------- phase 2: causal attention per head -------------
        # Scores computed TRANSPOSED (S^T[k,q]) so exp'd P^T feeds the PV
        # matmul directly -- no PE transposes of P.
        NHALF = 2 if NG >= 2 else 1
        HS = S // NHALF
        dpool = ctx.enter_context(tc.tile_pool(name="dramp", bufs=1,
                                               space="DRAM"))
        ag_in = [[dpool.tile([128, HS], DT.bfloat16,
                             name=f"ag_in{h}_{half}") for half in range(NHALF)]
                 for h in range(QH)]
        ag_out = [[dpool.tile([128 * N_CORES, HS], DT.bfloat16,
                              name=f"ag_out{h}_{half}", addr_space="Shared")
                   for half in range(NHALF)]
                  for h in range(QH)]

        def trigger_gather(h, half):
            hs = slice(half * HS, (half + 1) * HS)
            nc.scalar.dma_start(out=ag_in[h][half][:], in_=oT_sb[h][:, hs])
            nc.gpsimd.collective_compute(
                "AllGather", mybir.AluOpType.bypass,
                replica_groups=[list(range(N_CORES))],
                ins=[ag_in[h][half][:]], outs=[ag_out[h][half][:]])

        with ExitStack() as s2, nc.named_scope("phase2_attn"):
            st_pool = s2.enter_context(tc.tile_pool(name="stp", bufs=4,
                                                    space="PSUM"))
            ot_pool = s2.enter_context(tc.tile_pool(name="otp", bufs=4,
                                                    space="PSUM"))
            pt_pool = s2.enter_context(tc.tile_pool(name="ptp", bufs=6))
            racc_pool = s2.enter_context(tc.tile_pool(name="rap", bufs=3))
            rr_pool = s2.enter_context(tc.tile_pool(name="rrp", bufs=2))
            rsb_pool = s2.enter_context(tc.tile_pool(name="rsp", bufs=2))

            def finish_group(fin):
                """Tail of a q group: tree-reduce the row sums, reciprocal,
                broadcast via PE outer product, evict OT normalized, and
                trigger the half-head gather when ready.  Deferred so the
                PE work here hides behind the NEXT group's scores."""
                h, g, racc, OT = fin
                gs = slice(GW * g, GW * (g + 1))
                # row sums over the partition (k) axis on the idle GpSimd
                # engine (output replicated across partitions), then a
                # full-width reciprocal -- zero PE cycles, all DVE lanes
                rrep = rr_pool.tile([128, GW], DT.float32,
                                    name=f"rr{h}_{g}", tag="rr")
                nc.gpsimd.partition_all_reduce(rrep[:], racc[:], 128,
                                               bass_isa.ReduceOp.add)
                R_sb = rsb_pool.tile([128, GW], DT.float32,
                                     name=f"Rs{h}_{g}", tag="Rs")
                nc.vector.reciprocal(R_sb[:], rrep[:])
                # normalization folded into the OT eviction
                nc.vector.tensor_tensor(out=oT_sb[h][:, gs],
                                        in0=OT[:], in1=R_sb[:],
                                        op=mybir.AluOpType.mult)
                if NHALF == 2 and g == NG // 2 - 1:
                    trigger_gather(h, 0)
                elif g == NG - 1:
                    trigger_gather(h, NHALF - 1)

            prev_pv = {}      # h -> (OT, j, pt, is_last); crosses group bounds
            pending_fin = []

            def emit_pv(p):
                pOT, pj, ppt, poff, plast = p
                nc.tensor.matmul(pOT[:, poff:],
                                 v_sb[:, 128 * pj:128 * (pj + 1)],
                                 ppt[:, poff:],
                                 start=(pj == 0), stop=plast,
                                 skip_group_check=True)

            # heads processed in lockstep PAIRS: two independent
            # S->exp->PV chains interleave on every engine, so no engine
            # ever waits a full cross-engine round trip.  The causal mask
            # is added on the PE itself (identity-matmul accumulation into
            # the scores PSUM group) -- VectorE stays out of the chain.
            for hp in range(0, QH, 2):
                # flush the previous pair's trailing PV matmuls so their
                # OT groups are closed before finish_group reads them
                for h in [k for k in prev_pv if k not in (hp, hp + 1)]:
                    emit_pv(prev_pv.pop(h))
                for g in range(NG):
                    jmax = DV * g + DV
                    cur = []
                    for h in (hp, hp + 1):
                        racc = racc_pool.tile([128, GW], DT.float32,
                                              name=f"racc{h}_{g}",
                                              tag="racc")
                        OT = ot_pool.tile([128, GW], DT.float32,
                                          name=f"OT{h}_{g}", tag="OT")
                        cur.append((h, racc, OT))
                    gs = slice(GW * g, GW * (g + 1))
                    for j in range(jmax):
                        v_ = j - DV * g
                        # cols [0, 128*v_) of this chunk are strictly
                        # below the diagonal (q < k): skip them entirely
                        off = 128 * v_ if v_ > 0 else 0
                        for h, racc, OT in cur:
                            st = st_pool.tile([128, GW], DT.float32,
                                              name=f"st{h}_{g}_{j}",
                                              tag="st")
                            nc.tensor.matmul(
                                st[:, off:],
                                qk_sb[QH][:, 128 * j:128 * (j + 1)],
                                qk_sb[h][:, GW * g + off:GW * (g + 1)],
                                start=True, stop=(v_ < 0),
                                skip_group_check=True)
                            if v_ >= 0:
                                nc.tensor.matmul(
                                    st[:, off:off + 128], ident[:],
                                    maskTri[:],
                                    start=False, stop=True,
                                    skip_group_check=True)
                            pt = pt_pool.tile([128, GW], DT.bfloat16,
                                              name=f"pt{h}_{g}_{j}",
                                              tag="pt")
                            nc.scalar.activation(
                                pt[:, off:], st[:, off:],
                                mybir.ActivationFunctionType.Exp,
                                bias=0.0, scale=SCALE)
                            if h in prev_pv:
                                emit_pv(prev_pv.pop(h))
                            if j == 0:
                                nc.vector.tensor_copy(racc[:], pt[:])
                            else:
                                nc.vector.tensor_tensor(
                                    out=racc[:, off:], in0=racc[:, off:],
                                    in1=pt[:, off:],
                                    op=mybir.AluOpType.add)
                            prev_pv[h] = (OT, j, pt, off, j == jmax - 1)
                        if j == 1 and pending_fin:
                            for fin in pending_fin:
                                finish_group(fin)
                            pending_fin = []
                    for h, racc, OT in cur:
                        pending_fin.append((h, g, racc, OT))
            for h in list(prev_pv):
                emit_pv(prev_pv.pop(h))
            for fin in pending_fin:
                finish_group(fin)

        # ---------------- phase 3: output projection ----------------------
        with ExitStack() as s3, nc.named_scope("phase3_oproj"):
            agp = s3.enter_context(tc.tile_pool(name="agp", bufs=8))
            acc3 = s3.enter_context(tc.tile_pool(name="acc3", bufs=2,
                                                 space="PSUM"))
            osb = s3.enter_context(tc.tile_pool(name="osb", bufs=2))

            # gathered tiles: issue heads 0..QH-2 DMAs for all chunks
            # first; the last head's DMAs (which wait on the final
            # gathers) go last so they never head-of-line block the queue
            at_tiles = {}

            def issue_at_dma(cb, h):
                ns = slice(CH * cb, CH * (cb + 1))
                at = agp.tile([128, N_CORES * CH], DT.bfloat16,
                              name=f"ag_{cb}_{h}", tag="ag")
                half = 0 if CH * cb < HS else 1
                hns = slice(CH * cb - half * HS, CH * (cb + 1) - half * HS)
                nc.sync.dma_start(
                    out=at[:].rearrange("p (r n) -> p r n", r=N_CORES),
                    in_=ag_out[h][half][:, hns].rearrange(
                        "(r p) n -> p r n", p=128))
                at_tiles[(cb, h)] = at

            # DMA issue order follows gather completion order (all heads'
            # half-0 gathers land before any half-1): no head-of-line
            # blocking in the DMA queue
            for half in range(NHALF):
                for h in range(QH):
                    for cb in range(NCH):
                        if CH * cb // HS == half:
                            issue_at_dma(cb, h)

            # m-sequential accumulation: one PSUM bank at a time, each
            # m's eviction + output DMA overlaps the next m's matmuls
            for cb in range(NCH):
                ns = slice(CH * cb, CH * (cb + 1))
                for m in range(QH):
                    acc = acc3.tile([128, CH], DT.float32,
                                    name=f"acc3_{cb}_{m}", tag="a3")
                    ki = 0
                    for h in range(QH):
                        at = at_tiles[(cb, h)]
                        for r in range(N_CORES):
                            krow = QH * r + h  # global contraction block
                            nc.tensor.matmul(acc[:],
                                             wo_big[:, (krow * QH + m) * 128:
                                                       (krow * QH + m + 1) * 128],
                                             at[:, r * CH:(r + 1) * CH],
                                             start=(ki == 0),
                                             stop=(ki == QH * N_CORES - 1))
                            ki += 1
                    ob = osb.tile([128, CH], DT.float32,
                                  name=f"o3_{cb}_{m}", tag="o3")
                    nc.scalar.copy(ob[:], acc[:])
                    nc.sync.dma_start(out=out[128 * m:128 * (m + 1), ns],
                                      in_=ob[:])

    nc.compile()
    return nc


def host_inputs(x, wq, wk, wv, wo, S=2048):
    """Shard + preprocess full inputs into per-core input maps."""
    bf16 = ml_dtypes.bfloat16
    xT = np.ascontiguousarray(x.reshape(S, H).T).astype(bf16)
    inv_freq = 1.0 / (500000.0 ** (np.arange(0, HD, 2, dtype=np.float32) / HD))
    t = np.arange(S, dtype=np.float32)
    emb = np.concatenate([np.outer(t, inv_freq)] * 2, axis=-1)  # [S, HD]
    cosT = np.ascontiguousarray(np.cos(emb).T).astype(np.float32)
    sinT = np.ascontiguousarray(np.sin(emb).T).astype(np.float32)
    sinTs = sinT.copy()
    sinTs[0:64] = -sinTs[0:64]  # sign-folded for the rotate_half add
    in_maps = []
    for c in range(N_CORES):
        wqkv = np.concatenate([
            wq[128 * QH * c:128 * QH * (c + 1)],
            wk[HD * c:HD * (c + 1)],
            wv[HD * c:HD * (c + 1)],
        ], axis=0)  # [768, H]
        wqkvT = np.ascontiguousarray(wqkv.T).astype(bf16)
        woT = np.ascontiguousarray(
            wo[128 * QH * c:128 * QH * (c + 1)].T).astype(bf16)
        in_maps.append({
            "xT": xT, "wqkvT": wqkvT, "woT": woT,
            "cosT": cosT, "sinTs": sinTs,
        })
    return in_maps


_NC_CACHE = {}


def _get_nc(S=2048):
    if S not in _NC_CACHE:
        _NC_CACHE[S] = build_nc(S)
    return _NC_CACHE[S]


def run(inputs, S=2048, trace=False):
    nc = _get_nc(S)
    in_maps = host_inputs(inputs["x"], inputs["wq"], inputs["wk"],
                          inputs["wv"], inputs["wo"], S=S)
    res = run_bass_kernel_spmd(nc, in_maps, list(range(N_CORES)),
                               trace=trace)
    outp = np.empty((1, S, H), dtype=np.float32)
    for c in range(N_CORES):
        outp[0, :, 128 * QH * c:128 * QH * (c + 1)] = res.results[c]["out"].T
    return outp, res


def kernel(**inputs):
    outp, _ = run(inputs, S=2048, trace=False)
    return outp
